# revision 7
# baseline (speedup 1.0000x reference)
"""Trainium2 Bass kernel for nn_LCNNConvolution (GNN message passing).

Math:  out[n] = sum_p softplus( gather(X, NS[n,p,:]).flat @ W.T + b ) - 12*ln2
Key transform: W is block-structured over the 8 neighbor slots, so
    x1[n,p,:] = sum_k Y_k[NS[n,p,k]]        with  Y_k = X @ W_k.T  (+ b/1 baked
into slot 7). We precompute Y on-chip (PE matmul, fp16 in / f32 psum), write it
to DRAM as [row, 8*64] f32, then the hot loop is an indirect-DMA gather of
256B rows + DVE reduction over the 8 slots + ACT softplus + DVE reduction over
the 12 perms.

Host->device transfer over the axon tunnel is the wall-clock bottleneck, so
each core ships ONE consolidated fp16 tensor (~2 MB):
  [:, :25088]        X.T stripe (16 of 128 rows)    - AllGather'd on device
  [:, 25088:25600]   W stripe (16 of 128 rows)      - AllGather'd on device
  [:, 25600:25664]   bias (row 0 of core 0 only)    - AllGather'd on device
  [:, 25664:64064]   int16 gather codes as f16 bits - per-core private
Gather codes: ONE int16 per (site, perm, slot) in the 16-partition-wrapped
layout the gather engine wants; code = int16 bits of (site+1). The two gather
banks are derived ON DEVICE:
    idxA = max(code, 0)                      (bank A rows: [Z, X[0..32766]])
    idxB = mult(code+16385+16384, code<0)    (bank B rows: [Z2, X[32767..]];
                                              double-add is overflow-safe
                                              under wrap OR saturate ALUs)
and the 16->128 partition replication the gather engine expects is one
broadcast DMA per chunk. Output ships back as fp16.

Sharding: data-parallel over sites; each of the 8 cores handles 6250 sites and
computes its own full Y copy (X / W replicated on device via the AllGather).

Execution: the axon run path re-traces + re-lowers a fresh jax.jit closure on
every run_bass_kernel_spmd call (~0.5 s); `_run_cached` performs the identical
bass2jax PJRT execution but caches the jitted executable across calls.
run_bass_kernel_spmd remains the fallback path.
"""

import numpy as np

import concourse.bass as bass
import concourse.bacc as bacc
import concourse.mybir as mybir
import concourse.tile as tile
from concourse.bass_utils import run_bass_kernel_spmd

# ---------------------------------------------------------------- constants
N_SITES = 50000
NODE_F = 64
N_PERM = 12
N_NEIGH = 8
OUT_F = 64
LN2 = float(np.log(2.0))

N_CORES = 8
SITES_PER_CORE = N_SITES // N_CORES            # 6250
SITES_PER_PART = 50                            # ceil(6250/128) padded to 50
PAD_SITES = 128 * SITES_PER_PART               # 6400
COLS = SITES_PER_PART * N_PERM                 # 600 rows (n,p) per partition
GCOLS = 8                                      # cols per dma_gather call
N_CHUNKS = COLS // GCOLS                       # 75 gather chunks
NIDX = 128 * GCOLS                             # 1024 gathers/call
RCOLS = 24                                     # cols per reduce group (2 sites)
BANK = 32768                                   # bank A: ybig rows [0, 32768)
YROWS = N_SITES + 2                            # Z + 32767 sites + Z2 + rest

XT_HALF = 25088                                # 196*128, top half site count
WT_OFF = XT_HALF                               # 25088
BZ_OFF = XT_HALF + 512                         # 25600
AG_COLS = XT_HALF + 512 + 64                   # 25664 allgathered region
IDX_COLS = N_CHUNKS * 512                      # 38400
IN_COLS = AG_COLS + IDX_COLS                   # 64064

F32 = mybir.dt.float32
F16 = mybir.dt.float16
I16 = mybir.dt.int16


# ---------------------------------------------------------------- device IR
def build_nc():
    nc = bacc.Bacc("TRN2", target_bir_lowering=False, debug=False, num_devices=8)

    xtwi = nc.dram_tensor("xtwi", [16, IN_COLS], F16, kind="ExternalInput").ap()
    out = nc.dram_tensor(
        "out", [128, SITES_PER_PART, OUT_F], F16, kind="ExternalOutput"
    ).ap()

    with tile.TileContext(nc) as tc:
        with (
            tc.tile_pool(name="persist", bufs=1) as persist,
            tc.tile_pool(name="dram", bufs=1, space="DRAM") as dram,
        ):
            half_sb = persist.tile([128, 1], F32)
            nc.vector.memset(half_sb[:], 0.5)

            # replicate X / W / b across cores from the 16-row stripes
            ag_in = dram.tile([16, AG_COLS], F16)
            ag_full = dram.tile([128, AG_COLS], F16)
            nc.gpsimd.dma_start(ag_in[:], xtwi[:, 0:AG_COLS])
            nc.gpsimd.collective_compute(
                "AllGather",
                mybir.AluOpType.bypass,
                replica_groups=[list(range(N_CORES))],
                ins=[ag_in.opt()],
                outs=[ag_full.opt()],
            )

            # rows: [Z, sites 0..32766, Z2, sites 32767..49999]
            ybig = dram.tile([YROWS, 512], F32)
            zrow = persist.tile([1, 512], F32)
            nc.vector.memset(zrow[:], 0.0)
            nc.sync.dma_start(out=ybig[0:1, :], in_=zrow[:])
            nc.sync.dma_start(out=ybig[BANK : BANK + 1, :], in_=zrow[:])

            # ---------------- phase 1: Y = X @ Wall.T  (+bias in slot 7)
            with (
                tc.tile_pool(name="p1", bufs=1) as p1,
                tc.tile_pool(name="p1y", bufs=4) as p1y,
                tc.tile_pool(name="p1ps", bufs=4, space="PSUM") as p1ps,
            ):
                xt_sb = p1.tile([128, XT_HALF], F16)
                nc.sync.dma_start(out=xt_sb[:], in_=ag_full[:, 0:XT_HALF])
                wt_sb = p1.tile([128, 512], F16)
                nc.sync.dma_start(out=wt_sb[:], in_=ag_full[:, WT_OFF : WT_OFF + 512])
                bz_sb = p1.tile([1, 64], F16)
                nc.sync.dma_start(out=bz_sb[:], in_=ag_full[0:1, BZ_OFF : BZ_OFF + 64])
                ones_sb = p1.tile([1, 128], F16)
                nc.vector.memset(ones_sb[:], 1.0)

                for h in range(2):
                    for j in range(XT_HALF // 128):
                        s0 = h * XT_HALF + j * 128  # first site of this block
                        if s0 >= N_SITES:
                            break
                        nrows = min(128, N_SITES - s0)
                        psum = p1ps.tile([128, 512], F32, space="PSUM", tag="ps")
                        lhsT = xt_sb[64 * h : 64 * h + 64, j * 128 : (j + 1) * 128]
                        nc.tensor.matmul(
                            out=psum[:, 0:448],
                            lhsT=lhsT,
                            rhs=wt_sb[64 * h : 64 * h + 64, 0:448],
                            start=True,
                            stop=True,
                        )
                        nc.tensor.matmul(
                            out=psum[:, 448:512],
                            lhsT=lhsT,
                            rhs=wt_sb[64 * h : 64 * h + 64, 448:512],
                            start=True,
                            stop=False,
                        )
                        nc.tensor.matmul(
                            out=psum[:, 448:512],
                            lhsT=ones_sb[:1, :128],
                            rhs=bz_sb[:1, :64],
                            start=False,
                            stop=True,
                        )
                        y_sb = p1y.tile([128, 512], F32, tag="y")
                        nc.scalar.copy(out=y_sb[:], in_=psum[:])
                        # site s -> ybig row 1+s (s<=32766) / 2+s (s>=32767)
                        lo = min(nrows, max(0, 32767 - s0))
                        if lo > 0:
                            nc.sync.dma_start(
                                out=ybig[1 + s0 : 1 + s0 + lo, :],
                                in_=y_sb[:lo, :],
                            )
                        if lo < nrows:
                            nc.sync.dma_start(
                                out=ybig[2 + s0 + lo : 2 + s0 + nrows, :],
                                in_=y_sb[lo:nrows, :],
                            )

            # ---------------- phase 2: dma_gather (2 banks x 8 slots) + reduce
            # For each output row r and slot k: site s gathered from ybig via
            # bank A (code>0: row code, else row 0 = Z) and bank B (code<0:
            # row code+32769 wrapped, else row 0 = Z2). x1 = sum of all 16.
            with (
                tc.tile_pool(name="p2g", bufs=2) as p2g,
                tc.tile_pool(name="p2i", bufs=2) as p2i,
                tc.tile_pool(name="p2o", bufs=2) as p2o,
            ):
                x1 = None
                for j in range(N_CHUNKS):
                    raw = p2i.tile([128, 512], I16, tag="raw")
                    src = xtwi[:, AG_COLS + j * 512 : AG_COLS + (j + 1) * 512]
                    nc.sync.dma_start(
                        out=raw[:],
                        in_=src.bitcast(I16).unsqueeze(0).broadcast_to((8, 16, 512)),
                    )
                    idxa = p2i.tile([128, 512], I16, tag="idxa")
                    nc.vector.tensor_scalar(
                        out=idxa[:], in0=raw[:], scalar1=0, scalar2=None,
                        op0=mybir.AluOpType.max,
                    )
                    m = p2i.tile([128, 512], I16, tag="m")
                    nc.vector.tensor_scalar(
                        out=m[:], in0=raw[:], scalar1=0, scalar2=None,
                        op0=mybir.AluOpType.is_lt,
                    )
                    t2 = p2i.tile([128, 512], I16, tag="t2")
                    nc.vector.tensor_scalar(
                        out=t2[:], in0=raw[:], scalar1=16385, scalar2=16384,
                        op0=mybir.AluOpType.add, op1=mybir.AluOpType.add,
                    )
                    idxb = p2i.tile([128, 512], I16, tag="idxb")
                    nc.vector.tensor_tensor(
                        out=idxb[:], in0=t2[:], in1=m[:],
                        op=mybir.AluOpType.mult,
                    )
                    g = p2g.tile([128, 16, GCOLS, OUT_F], F32, tag="g")
                    for k in range(8):
                        nc.gpsimd.dma_gather(
                            out_ap=g[:, k, :, :],
                            in_ap=ybig[0:BANK, k * 64 : (k + 1) * 64],
                            idxs_ap=idxa[:, k * 64 : (k + 1) * 64],
                            num_idxs=NIDX,
                            num_idxs_reg=NIDX,
                            elem_size=64,
                            elem_step=512,
                        )
                        nc.gpsimd.dma_gather(
                            out_ap=g[:, 8 + k, :, :],
                            in_ap=ybig[BANK:YROWS, k * 64 : (k + 1) * 64],
                            idxs_ap=idxb[:, k * 64 : (k + 1) * 64],
                            num_idxs=NIDX,
                            num_idxs_reg=NIDX,
                            elem_size=64,
                            elem_step=512,
                        )
                    # x1[p, c, f] = sum_kb g[p, kb, c, f] into its 8-col slice
                    if j % 3 == 0:
                        x1 = p2o.tile([128, RCOLS, OUT_F], F32, tag="x1")
                    sub = j % 3
                    nc.vector.tensor_reduce(
                        out=x1[:, sub * GCOLS : (sub + 1) * GCOLS, :],
                        in_=g[:].rearrange("p k c f -> p c f k"),
                        axis=mybir.AxisListType.X,
                        op=mybir.AluOpType.add,
                    )
                    if sub != 2:
                        continue
                    grp = j // 3  # 24-col group = 2 sites
                    # softplus(x) - ln2 == Ln(0.5*Exp(x) + 0.5)
                    x2 = p2o.tile([128, RCOLS, OUT_F], F32, tag="x2")
                    nc.scalar.activation(
                        out=x2[:],
                        in_=x1[:],
                        func=mybir.ActivationFunctionType.Exp,
                    )
                    nc.scalar.activation(
                        out=x2[:],
                        in_=x2[:],
                        func=mybir.ActivationFunctionType.Ln,
                        scale=0.5,
                        bias=half_sb[:],
                    )
                    # out[p, s, f] = sum_q x2[p, s*12+q, f]
                    acc = p2o.tile([128, RCOLS // N_PERM, OUT_F], F32, tag="acc")
                    nc.vector.tensor_reduce(
                        out=acc[:],
                        in_=x2[:].rearrange("p (s q) f -> p s f q", q=N_PERM),
                        axis=mybir.AxisListType.X,
                        op=mybir.AluOpType.add,
                    )
                    acc16 = p2o.tile([128, RCOLS // N_PERM, OUT_F], F16, tag="a16")
                    nc.scalar.copy(out=acc16[:], in_=acc[:])
                    nc.sync.dma_start(
                        out=out[:, grp * 2 : grp * 2 + 2, :],
                        in_=acc16[:],
                    )

    nc.compile()
    return nc


# ---------------------------------------------------------------- host side
def _host_prep(X_sites, X_NSs, W, b):
    X_sites = np.asarray(X_sites, dtype=np.float32)
    X_NSs = np.asarray(X_NSs)
    W = np.asarray(W, dtype=np.float32)
    b = np.asarray(b, dtype=np.float32)

    bufs = np.zeros((N_CORES, 16, IN_COLS), dtype=np.float16)
    flat = bufs.reshape(128, IN_COLS)

    # X.T in two 64-row halves (casts f32->f16 during assignment, no temps)
    flat[:64, 0:XT_HALF] = X_sites[:XT_HALF].T
    flat[64:, 0 : N_SITES - XT_HALF] = X_sites[XT_HALF:].T

    wt = W.reshape(OUT_F, N_NEIGH, NODE_F).transpose(2, 1, 0).reshape(NODE_F, 512)
    flat[:64, WT_OFF : WT_OFF + 512] = wt
    flat[64:, WT_OFF : WT_OFF + 512] = wt
    flat[0, BZ_OFF : BZ_OFF + 64] = b

    # code = int16 bits of (site+1); padding sites get code 0 (-> Z + Z2)
    code = np.asarray(X_NSs).astype(np.uint16)
    code += 1
    codes = np.zeros((N_CORES, PAD_SITES, N_PERM, N_NEIGH), np.int16)
    codes[:, :SITES_PER_CORE] = code.view(np.int16).reshape(
        N_CORES, SITES_PER_CORE, N_PERM, N_NEIGH
    )
    # C[c, p, col, k], col = sp*12 + q;  gather position i = gcol*128 + p
    C = codes.reshape(N_CORES, 128, N_CHUNKS, GCOLS, N_NEIGH)
    arr = C.transpose(0, 2, 4, 3, 1).reshape(N_CORES, N_CHUNKS, N_NEIGH, NIDX)
    # 16-partition wrap: t16[c, j, k, scol, p_row] = arr[c, j, k, scol*16+p_row]
    t16 = arr.reshape(N_CORES, N_CHUNKS, N_NEIGH, NIDX // 16, 16)
    # idxf[c, p_row, j*512 + k*64 + scol]
    idxf = np.ascontiguousarray(t16.transpose(0, 4, 1, 2, 3)).reshape(
        N_CORES, 16, IDX_COLS
    )
    bufs[:, :, AG_COLS:] = idxf.view(np.float16)
    return [{"xtwi": bufs[c]} for c in range(N_CORES)]


_NC_CACHE = {}


def _get_nc():
    if "nc" not in _NC_CACHE:
        _NC_CACHE["nc"] = build_nc()
    return _NC_CACHE["nc"]


def _make_cached_runner(nc):
    """The same bass2jax PJRT execution run_bass_kernel_spmd performs under
    axon, but with the jitted shard_map executable cached so repeat calls
    skip the ~0.5 s re-trace/re-lower. Returns None if anything about the
    environment doesn't match expectations (caller falls back)."""
    import jax
    import jax.core
    from jax.experimental.shard_map import shard_map
    from jax.sharding import Mesh, PartitionSpec

    from concourse import bass2jax

    if nc.dbg_callbacks or nc.dbg_addr is not None:
        return None
    bass2jax.install_neuronx_cc_hook()

    partition_name = nc.partition_id_tensor.name if nc.partition_id_tensor else None
    in_names, out_names, out_avals, zero_shapes = [], [], [], []
    for alloc in nc.m.functions[0].allocations:
        if not isinstance(alloc, mybir.MemoryLocationSet):
            continue
        name = alloc.memorylocations[0].name
        if alloc.kind == "ExternalInput":
            if name != partition_name:
                in_names.append(name)
        elif alloc.kind == "ExternalOutput":
            shape = tuple(alloc.tensor_shape)
            dtype = mybir.dt.np(alloc.dtype)
            out_names.append(name)
            out_avals.append(jax.core.ShapedArray(shape, dtype))
            zero_shapes.append((shape, dtype))
    n_params = len(in_names)
    n_outs = len(out_avals)
    all_names = in_names + out_names
    if partition_name is not None:
        all_names.append(partition_name)
    donate = tuple(range(n_params, n_params + n_outs))

    def _body(*args):
        operands = list(args)
        if partition_name is not None:
            operands.append(bass2jax.partition_id_tensor())
        outs = bass2jax._bass_exec_p.bind(
            *operands,
            out_avals=tuple(out_avals),
            in_names=tuple(all_names),
            out_names=tuple(out_names),
            lowering_input_output_aliases=(),
            sim_require_finite=True,
            sim_require_nnan=True,
            nc=nc,
        )
        return tuple(outs)

    devices = jax.devices()[:N_CORES]
    if len(devices) < N_CORES:
        return None
    mesh = Mesh(np.asarray(devices), ("core",))
    in_specs = (PartitionSpec("core"),) * (n_params + n_outs)
    out_specs = (PartitionSpec("core"),) * n_outs
    sharded = jax.jit(
        shard_map(
            _body, mesh=mesh, in_specs=in_specs, out_specs=out_specs, check_rep=False
        ),
        donate_argnums=donate,
        keep_unused=True,
    )

    # The donated output buffers just need to exist on device with the right
    # shape/sharding — the kernel writes every output element. Upload zeros
    # once, then recycle each call's (already fetched) device outputs as the
    # next call's donated scratch, skipping the per-call host upload.
    donate_next = [None]

    def run(in_maps):
        concat_in = [
            np.concatenate([np.asarray(m[name]) for m in in_maps], axis=0)
            for name in in_names
        ]
        scratch = donate_next[0]
        donate_next[0] = None  # never reuse once handed over / after a failure
        if scratch is None:
            scratch = [
                np.zeros((N_CORES * s[0], *s[1:]), d) for (s, d) in zero_shapes
            ]
        out_arrs = sharded(*concat_in, *scratch)
        host = [np.asarray(a) for a in out_arrs]
        donate_next[0] = list(out_arrs)
        return [
            {
                name: host[i].reshape(N_CORES, *out_avals[i].shape)[c]
                for i, name in enumerate(out_names)
            }
            for c in range(N_CORES)
        ]

    return run


def _run(nc, in_maps):
    if "runner" not in _NC_CACHE:
        try:
            _NC_CACHE["runner"] = _make_cached_runner(nc)
        except Exception:
            _NC_CACHE["runner"] = None
    runner = _NC_CACHE["runner"]
    if runner is not None:
        try:
            return runner(in_maps)
        except Exception:
            pass  # fall back for this call; keep the runner for the next
    return run_bass_kernel_spmd(
        nc, in_maps, core_ids=list(range(N_CORES))
    ).results


def _stitch(results):
    full = np.empty((N_SITES, OUT_F), dtype=np.float32)
    for c, r in enumerate(results):
        o = r["out"].reshape(PAD_SITES, OUT_F)[:SITES_PER_CORE]
        full[c * SITES_PER_CORE : (c + 1) * SITES_PER_CORE] = o.astype(np.float32)
    return full


def kernel(X_sites, X_NSs, W, b, _trace=False):
    nc = _get_nc()
    in_maps = _host_prep(X_sites, X_NSs, W, b)
    if _trace:
        res = run_bass_kernel_spmd(
            nc, in_maps, core_ids=list(range(N_CORES)), trace=True
        )
        return _stitch(res.results), res
    return _stitch(_run(nc, in_maps))
